# revision 7
# baseline (speedup 1.0000x reference)
"""DSAttention (de-stationary attention) Trainium2 Bass kernel.

Reference semantics (per batch b, head h):
    scores[l, s] = (q[l] . k[s]) * tau[b] + delta[b, s]
    attn = softmax(scale * scores, axis=s),  scale = 1/sqrt(D)
    out[l] = sum_s attn[l, s] * v[s]

Sharding: B*H = 32 (b,h) pairs split 4-per-core across 8 NeuronCores.
Host folds tau[b]*scale into Q and scale into delta, and pre-transposes
Q/K to [D, L] so the device only does contiguous DMA loads.

Device algorithm per (b,h)  (flash-style, no max subtraction needed:
|logits| <= ~10 so exp() is safe in fp32):
    scoresT[s_blk, l] = K^T_blk.T @ Q^T           (PE, f32r)
    P = exp(scoresT + delta[s])                   (ACT, fused bias)
    outT[65, l]   += V'_blk.T @ P_blk             (PE, f32r accumulate)
      where V' = [V | ones]  ->  row 64 of outT = softmax denominator
    out[l, d] = transpose(outT)[:, 0:64] * (1/transpose(outT)[:, 64])
"""

import sys
from contextlib import ExitStack

import numpy as np

sys.path.insert(0, "/opt/trn_rl_repo")

import concourse.bacc as bacc  # noqa: E402
import concourse.mybir as mybir  # noqa: E402
import concourse.tile as tile  # noqa: E402
from concourse import bass_utils  # noqa: E402

B, L, S, H, D = 4, 2048, 2048, 8, 64
NCORES = 8
BH = B * H
PER = BH // NCORES  # (b,h) pairs per core
SCALE = 1.0 / np.sqrt(D)
PB = 128  # partition block (s-chunk size)
F32 = mybir.dt.float32
F32R = mybir.dt.float32r

ActFn = mybir.ActivationFunctionType


def build_program(per=PER, l_len=L, s_len=S, d=D, w=1024):
    """Build the per-core Bass program. w = l-pass width (free-dim chunk)."""
    assert l_len % w == 0 and w % 512 == 0 and s_len % PB == 0
    ns = s_len // PB  # number of s-chunks
    npass = l_len // w  # l passes
    nj = w // 512  # 512-wide matmul slices per pass

    nc = bacc.Bacc("TRN2", target_bir_lowering=False, debug=False)

    qT_d = nc.dram_tensor("qT", [per, d, l_len], F32R, kind="ExternalInput").ap()
    kT_d = nc.dram_tensor("kT", [per, d, s_len], F32R, kind="ExternalInput").ap()
    v_d = nc.dram_tensor("v", [per, s_len, d + 1], F32R, kind="ExternalInput").ap()
    dl_d = nc.dram_tensor("dl", [per, s_len], F32, kind="ExternalInput").ap()
    id_d = nc.dram_tensor("ident", [d + 1, d + 1], F32, kind="ExternalInput").ap()
    o_d = nc.dram_tensor("out", [per, l_len, d], F32, kind="ExternalOutput").ap()

    with tile.TileContext(nc) as tc, ExitStack() as ctx:
        const_pool = ctx.enter_context(tc.tile_pool(name="const", bufs=1))
        in_pool = ctx.enter_context(tc.tile_pool(name="inp", bufs=2))
        p_pool = ctx.enter_context(tc.tile_pool(name="pp", bufs=3))
        ot_pool = ctx.enter_context(tc.tile_pool(name="otp", bufs=2))
        os_pool = ctx.enter_context(tc.tile_pool(name="osp", bufs=3))
        rc_pool = ctx.enter_context(tc.tile_pool(name="rcp", bufs=3))
        qk_psum = ctx.enter_context(tc.tile_pool(name="qkp", bufs=2, space="PSUM"))
        pv_psum = ctx.enter_context(tc.tile_pool(name="pvp", bufs=nj, space="PSUM"))
        tr_psum = ctx.enter_context(tc.tile_pool(name="trp", bufs=2, space="PSUM"))

        ident = const_pool.tile([d + 1, d + 1], F32)
        nc.sync.dma_start(ident[:], id_d[:])

        for i in range(per):
            qT = in_pool.tile([d, l_len], F32R, tag="qT")
            nc.sync.dma_start(qT[:], qT_d[i])
            kT = in_pool.tile([d, s_len], F32R, tag="kT")
            nc.sync.dma_start(kT[:], kT_d[i])

            # V' = [V | ones] (ones appended host-side) as [128, ns*(d+1)]
            vv = in_pool.tile([PB, ns * (d + 1)], F32R, tag="vv")
            vv3 = vv[:].rearrange("p (c e) -> p c e", e=d + 1)
            nc.sync.dma_start(
                vv3[:], v_d[i].rearrange("(c p) e -> p c e", p=PB)
            )

            # delta (pre-scaled) as per-partition bias columns [128, ns]
            dl = in_pool.tile([PB, ns], F32, tag="dl")
            nc.sync.dma_start(dl[:], dl_d[i].rearrange("(c p) -> p c", p=PB))

            for lp in range(npass):
                l0 = lp * w
                pvs = [
                    pv_psum.tile([d + 1, 512], F32, tag="pv", name=f"pv_{i}_{lp}_{j}")
                    for j in range(nj)
                ]
                for s in range(ns):
                    qk = qk_psum.tile([PB, w], F32, tag="qk")
                    for j in range(nj):
                        nc.tensor.matmul(
                            qk[:, j * 512 : (j + 1) * 512],
                            kT[:, s * PB : (s + 1) * PB],
                            qT[:, l0 + j * 512 : l0 + (j + 1) * 512],
                            start=True,
                            stop=True,
                        )
                    pt = p_pool.tile([PB, w], F32R, tag="p")
                    nc.scalar.activation(
                        pt[:], qk[:], ActFn.Exp, bias=dl[:, s : s + 1], scale=1.0
                    )
                    for j in range(nj):
                        nc.tensor.matmul(
                            pvs[j][:],
                            vv[:, s * (d + 1) : (s + 1) * (d + 1)],
                            pt[:, j * 512 : (j + 1) * 512],
                            start=(s == 0),
                            stop=(s == ns - 1),
                        )
                # epilogue: transpose [65, 128] blocks -> [128, 65], normalize
                ot = ot_pool.tile([d + 1, w], F32, tag="ot")
                for j in range(nj):
                    nc.vector.tensor_copy(ot[:, j * 512 : (j + 1) * 512], pvs[j][:])
                for j in range(w // PB):
                    tr = tr_psum.tile([PB, d + 1], F32, tag="tr")
                    nc.tensor.transpose(
                        tr[:], ot[:, j * PB : (j + 1) * PB], ident[:]
                    )
                    rc = rc_pool.tile([PB, 1], F32, tag="rc")
                    nc.vector.reciprocal(rc[:], tr[:, d : d + 1])
                    ob = os_pool.tile([PB, d], F32, tag="ob")
                    nc.vector.tensor_scalar_mul(ob[:], tr[:, 0:d], rc[:])
                    nc.sync.dma_start(
                        o_d[i, l0 + j * PB : l0 + (j + 1) * PB, :], ob[:]
                    )

    nc.compile()
    return nc


def prep_inputs(queries, keys, values, tau, delta):
    """Host-side shard prep: fold tau*scale into Q, scale delta, transpose
    Q/K to [D, L] layout, split (b,h) pairs across cores."""
    q = np.asarray(queries, dtype=np.float32)
    k = np.asarray(keys, dtype=np.float32)
    v = np.asarray(values, dtype=np.float32)
    tau = np.asarray(tau, dtype=np.float32)
    delta = np.asarray(delta, dtype=np.float32)

    qs = q * (SCALE * tau)[:, None, None, None]  # [B, L, H, D]
    qT = np.ascontiguousarray(qs.transpose(0, 2, 3, 1)).reshape(BH, D, L)
    kT = np.ascontiguousarray(k.transpose(0, 2, 3, 1)).reshape(BH, D, S)
    vb = np.ascontiguousarray(v.transpose(0, 2, 1, 3)).reshape(BH, S, D)
    vb = np.concatenate([vb, np.ones((BH, S, 1), dtype=np.float32)], axis=2)
    dls = (SCALE * delta).astype(np.float32)  # [B, S]
    ident = np.eye(D + 1, dtype=np.float32)

    in_maps = []
    for c in range(NCORES):
        sl = slice(c * PER, (c + 1) * PER)
        bh_idx = np.arange(c * PER, (c + 1) * PER)
        in_maps.append(
            {
                "qT": np.ascontiguousarray(qT[sl]),
                "kT": np.ascontiguousarray(kT[sl]),
                "v": np.ascontiguousarray(vb[sl]),
                "dl": np.ascontiguousarray(dls[bh_idx // H]),
                "ident": ident,
            }
        )
    return in_maps


def assemble_output(results):
    out_bh = np.concatenate([r["out"] for r in results], axis=0)  # [32, L, D]
    out = out_bh.reshape(B, H, L, D).transpose(0, 2, 1, 3)  # [B, L, H, D]
    return np.ascontiguousarray(out).astype(np.float32)


_NC_CACHE = {}


def get_program():
    if "nc" not in _NC_CACHE:
        _NC_CACHE["nc"] = build_program()
    return _NC_CACHE["nc"]


class PjrtRunner:
    """Cached shard_map runner mirroring bass2jax.run_bass_via_pjrt, but
    reusable across calls (single jit closure) so repeat executions skip
    retracing and input re-transfer (for benchmarking)."""

    def __init__(self, nc):
        import jax
        from jax.sharding import Mesh, PartitionSpec
        from jax.experimental.shard_map import shard_map
        from concourse import bass2jax, mybir as _mybir

        bass2jax.install_neuronx_cc_hook()
        self.nc = nc
        self.jax = jax
        in_names, out_names, out_avals, zero_outs = [], [], [], []
        for alloc in nc.m.functions[0].allocations:
            if not isinstance(alloc, _mybir.MemoryLocationSet):
                continue
            name = alloc.memorylocations[0].name
            if alloc.kind == "ExternalInput":
                in_names.append(name)
            elif alloc.kind == "ExternalOutput":
                shape = tuple(alloc.tensor_shape)
                dtype = _mybir.dt.np(alloc.dtype)
                out_names.append(name)
                out_avals.append(jax.core.ShapedArray(shape, dtype))
                zero_outs.append((shape, dtype))
        part_name = nc.partition_id_tensor.name if nc.partition_id_tensor else None
        if part_name is not None:
            in_names = [n for n in in_names if n != part_name]
        self.in_names, self.out_names = in_names, out_names
        self.out_shapes = zero_outs
        n_params = len(in_names)
        all_names = in_names + out_names

        def _body(*args):
            operands = list(args)
            if part_name is not None:
                operands.append(bass2jax.partition_id_tensor())
            outs = bass2jax._bass_exec_p.bind(
                *operands,
                out_avals=tuple(out_avals),
                in_names=tuple(all_names + ([part_name] if part_name else [])),
                out_names=tuple(out_names),
                lowering_input_output_aliases=(),
                sim_require_finite=True,
                sim_require_nnan=True,
                nc=nc,
            )
            return tuple(outs)

        devices = jax.devices()[:NCORES]
        self.mesh = Mesh(np.asarray(devices), ("core",))
        n_out = len(out_names)
        in_specs = (PartitionSpec("core"),) * (n_params + n_out)
        out_specs = (PartitionSpec("core"),) * n_out
        donate = tuple(range(n_params, n_params + n_out))
        self.fn = jax.jit(
            shard_map(
                _body,
                mesh=self.mesh,
                in_specs=in_specs,
                out_specs=out_specs,
                check_rep=False,
            ),
            donate_argnums=donate,
            keep_unused=True,
        )

    def device_inputs(self, in_maps):
        """Concat per-core inputs on axis 0 and push to devices once."""
        import jax
        from jax.sharding import NamedSharding, PartitionSpec

        sh = NamedSharding(self.mesh, PartitionSpec("core"))
        arrs = []
        for name in self.in_names:
            cat = np.concatenate([np.asarray(m[name]) for m in in_maps], axis=0)
            arrs.append(jax.device_put(cat, sh))
        return arrs

    def device_zeros(self):
        import jax.numpy as jnp
        from jax.sharding import NamedSharding, PartitionSpec

        sh = NamedSharding(self.mesh, PartitionSpec("core"))
        zs = []
        for shape, dtype in self.out_shapes:
            gshape = (shape[0] * NCORES,) + tuple(shape[1:])
            zs.append(
                self.jax.jit(
                    lambda s=gshape, d=dtype: jnp.zeros(s, d), out_shardings=sh
                )()
            )
        return zs

    def __call__(self, dev_in, dev_zeros):
        outs = self.fn(*dev_in, *dev_zeros)
        return [np.asarray(o) for o in outs]

    def split_outputs(self, np_outs):
        results = [dict() for _ in range(NCORES)]
        for name, arr in zip(self.out_names, np_outs):
            per = arr.shape[0] // NCORES
            for c in range(NCORES):
                results[c][name] = arr[c * per : (c + 1) * per]
        return results


def get_runner():
    if "runner" not in _NC_CACHE:
        _NC_CACHE["runner"] = PjrtRunner(get_program())
    return _NC_CACHE["runner"]


def run(inputs):
    r = get_runner()
    in_maps = prep_inputs(**inputs)
    dev_in = r.device_inputs(in_maps)
    np_outs = r(dev_in, r.device_zeros())
    return assemble_output(r.split_outputs(np_outs)), r


def kernel(**inputs) -> np.ndarray:
    out, _ = run(inputs)
    return out


# revision 8
# speedup vs baseline: 251.3069x; 251.3069x over previous
"""DSAttention (de-stationary attention) Trainium2 Bass kernel.

Reference semantics (per batch b, head h):
    scores[l, s] = (q[l] . k[s]) * tau[b] + delta[b, s]
    attn = softmax(scale * scores, axis=s),  scale = 1/sqrt(D)
    out[l] = sum_s attn[l, s] * v[s]

Sharding: B*H = 32 (b,h) pairs split 4-per-core across 8 NeuronCores.
Host folds tau[b]*scale into Q and scale into delta, and pre-transposes
Q/K to [D, L] so the device only does contiguous DMA loads.

Device algorithm per (b,h)  (flash-style, no max subtraction needed:
|logits| <= ~10 so exp() is safe in fp32):
    scoresT[s_blk, l] = K^T_blk.T @ Q^T           (PE, f32r)
    P = exp(scoresT + delta[s])                   (ACT, fused bias)
    outT[65, l]   += V'_blk.T @ P_blk             (PE, f32r accumulate)
      where V' = [V | ones]  ->  row 64 of outT = softmax denominator
    out[l, d] = transpose(outT)[:, 0:64] * (1/transpose(outT)[:, 64])
"""

import sys
from contextlib import ExitStack

import numpy as np

sys.path.insert(0, "/opt/trn_rl_repo")

import concourse.bacc as bacc  # noqa: E402
import concourse.mybir as mybir  # noqa: E402
import concourse.tile as tile  # noqa: E402
from concourse import bass_utils  # noqa: E402

B, L, S, H, D = 4, 2048, 2048, 8, 64
NCORES = 8
BH = B * H
PER = BH // NCORES  # (b,h) pairs per core
SCALE = 1.0 / np.sqrt(D)
PB = 128  # partition block (s-chunk size)
F32 = mybir.dt.float32
F32R = mybir.dt.float32r

ActFn = mybir.ActivationFunctionType


def build_program(per=PER, l_len=L, s_len=S, d=D, w=1024, reps=1):
    """Build the per-core Bass program. w = l-pass width (free-dim chunk)."""
    assert l_len % w == 0 and w % 512 == 0 and s_len % PB == 0
    ns = s_len // PB  # number of s-chunks
    npass = l_len // w  # l passes
    nj = w // 512  # 512-wide matmul slices per pass

    nc = bacc.Bacc("TRN2", target_bir_lowering=False, debug=False)

    qT_d = nc.dram_tensor("qT", [per, d, l_len], F32R, kind="ExternalInput").ap()
    kT_d = nc.dram_tensor("kT", [per, d, s_len], F32R, kind="ExternalInput").ap()
    v_d = nc.dram_tensor("v", [per, s_len, d + 1], F32R, kind="ExternalInput").ap()
    dl_d = nc.dram_tensor("dl", [per, s_len], F32, kind="ExternalInput").ap()
    id_d = nc.dram_tensor("ident", [d + 1, d + 1], F32, kind="ExternalInput").ap()
    o_d = nc.dram_tensor("out", [per, l_len, d], F32, kind="ExternalOutput").ap()

    with tile.TileContext(nc) as tc, ExitStack() as ctx:
        const_pool = ctx.enter_context(tc.tile_pool(name="const", bufs=1))
        in_pool = ctx.enter_context(tc.tile_pool(name="inp", bufs=2))
        p_pool = ctx.enter_context(tc.tile_pool(name="pp", bufs=3))
        ot_pool = ctx.enter_context(tc.tile_pool(name="otp", bufs=2))
        os_pool = ctx.enter_context(tc.tile_pool(name="osp", bufs=3))
        rc_pool = ctx.enter_context(tc.tile_pool(name="rcp", bufs=3))
        qk_psum = ctx.enter_context(tc.tile_pool(name="qkp", bufs=2, space="PSUM"))
        pv_psum = ctx.enter_context(tc.tile_pool(name="pvp", bufs=nj, space="PSUM"))
        tr_psum = ctx.enter_context(tc.tile_pool(name="trp", bufs=2, space="PSUM"))

        ident = const_pool.tile([d + 1, d + 1], F32)
        nc.sync.dma_start(ident[:], id_d[:])

        rep_ctx = (
            tc.For_i(0, reps, 1, hint_engines=(mybir.EngineType.PE,))
            if reps > 1
            else None
        )
        if rep_ctx is not None:
            ctx.enter_context(rep_ctx)
        for i in range(per):
            qT = in_pool.tile([d, l_len], F32R, tag="qT")
            nc.sync.dma_start(qT[:], qT_d[i])
            kT = in_pool.tile([d, s_len], F32R, tag="kT")
            nc.sync.dma_start(kT[:], kT_d[i])

            # V' = [V | ones] (ones appended host-side) as [128, ns*(d+1)]
            vv = in_pool.tile([PB, ns * (d + 1)], F32R, tag="vv")
            vv3 = vv[:].rearrange("p (c e) -> p c e", e=d + 1)
            nc.sync.dma_start(
                vv3[:], v_d[i].rearrange("(c p) e -> p c e", p=PB)
            )

            # delta (pre-scaled) as per-partition bias columns [128, ns]
            dl = in_pool.tile([PB, ns], F32, tag="dl")
            nc.sync.dma_start(dl[:], dl_d[i].rearrange("(c p) -> p c", p=PB))

            for lp in range(npass):
                l0 = lp * w
                pvs = [
                    pv_psum.tile([d + 1, 512], F32, tag="pv", name=f"pv_{i}_{lp}_{j}")
                    for j in range(nj)
                ]
                for s in range(ns):
                    qk = qk_psum.tile([PB, w], F32, tag="qk")
                    for j in range(nj):
                        nc.tensor.matmul(
                            qk[:, j * 512 : (j + 1) * 512],
                            kT[:, s * PB : (s + 1) * PB],
                            qT[:, l0 + j * 512 : l0 + (j + 1) * 512],
                            start=True,
                            stop=True,
                        )
                    pt = p_pool.tile([PB, w], F32R, tag="p")
                    nc.scalar.activation(
                        pt[:], qk[:], ActFn.Exp, bias=dl[:, s : s + 1], scale=1.0
                    )
                    for j in range(nj):
                        nc.tensor.matmul(
                            pvs[j][:],
                            vv[:, s * (d + 1) : (s + 1) * (d + 1)],
                            pt[:, j * 512 : (j + 1) * 512],
                            start=(s == 0),
                            stop=(s == ns - 1),
                        )
                # epilogue: transpose [65, 128] blocks -> [128, 65], normalize
                ot = ot_pool.tile([d + 1, w], F32, tag="ot")
                for j in range(nj):
                    nc.vector.tensor_copy(ot[:, j * 512 : (j + 1) * 512], pvs[j][:])
                for j in range(w // PB):
                    tr = tr_psum.tile([PB, d + 1], F32, tag="tr")
                    nc.tensor.transpose(
                        tr[:], ot[:, j * PB : (j + 1) * PB], ident[:]
                    )
                    rc = rc_pool.tile([PB, 1], F32, tag="rc")
                    nc.vector.reciprocal(rc[:], tr[:, d : d + 1])
                    ob = os_pool.tile([PB, d], F32, tag="ob")
                    nc.vector.tensor_scalar_mul(ob[:], tr[:, 0:d], rc[:])
                    nc.sync.dma_start(
                        o_d[i, l0 + j * PB : l0 + (j + 1) * PB, :], ob[:]
                    )

    nc.compile()
    return nc


def prep_inputs(queries, keys, values, tau, delta):
    """Host-side shard prep: fold tau*scale into Q, scale delta, transpose
    Q/K to [D, L] layout, split (b,h) pairs across cores."""
    q = np.asarray(queries, dtype=np.float32)
    k = np.asarray(keys, dtype=np.float32)
    v = np.asarray(values, dtype=np.float32)
    tau = np.asarray(tau, dtype=np.float32)
    delta = np.asarray(delta, dtype=np.float32)

    qs = q * (SCALE * tau)[:, None, None, None]  # [B, L, H, D]
    qT = np.ascontiguousarray(qs.transpose(0, 2, 3, 1)).reshape(BH, D, L)
    kT = np.ascontiguousarray(k.transpose(0, 2, 3, 1)).reshape(BH, D, S)
    vb = np.ascontiguousarray(v.transpose(0, 2, 1, 3)).reshape(BH, S, D)
    vb = np.concatenate([vb, np.ones((BH, S, 1), dtype=np.float32)], axis=2)
    dls = (SCALE * delta).astype(np.float32)  # [B, S]
    ident = np.eye(D + 1, dtype=np.float32)

    in_maps = []
    for c in range(NCORES):
        sl = slice(c * PER, (c + 1) * PER)
        bh_idx = np.arange(c * PER, (c + 1) * PER)
        in_maps.append(
            {
                "qT": np.ascontiguousarray(qT[sl]),
                "kT": np.ascontiguousarray(kT[sl]),
                "v": np.ascontiguousarray(vb[sl]),
                "dl": np.ascontiguousarray(dls[bh_idx // H]),
                "ident": ident,
            }
        )
    return in_maps


def assemble_output(results):
    out_bh = np.concatenate([r["out"] for r in results], axis=0)  # [32, L, D]
    out = out_bh.reshape(B, H, L, D).transpose(0, 2, 1, 3)  # [B, L, H, D]
    return np.ascontiguousarray(out).astype(np.float32)


_NC_CACHE = {}


def get_program():
    if "nc" not in _NC_CACHE:
        _NC_CACHE["nc"] = build_program()
    return _NC_CACHE["nc"]


class PjrtRunner:
    """Cached shard_map runner mirroring bass2jax.run_bass_via_pjrt, but
    reusable across calls (single jit closure) so repeat executions skip
    retracing and input re-transfer (for benchmarking)."""

    def __init__(self, nc):
        import jax
        from jax.sharding import Mesh, PartitionSpec
        from jax.experimental.shard_map import shard_map
        from concourse import bass2jax, mybir as _mybir

        bass2jax.install_neuronx_cc_hook()
        self.nc = nc
        self.jax = jax
        in_names, out_names, out_avals, zero_outs = [], [], [], []
        for alloc in nc.m.functions[0].allocations:
            if not isinstance(alloc, _mybir.MemoryLocationSet):
                continue
            name = alloc.memorylocations[0].name
            if alloc.kind == "ExternalInput":
                in_names.append(name)
            elif alloc.kind == "ExternalOutput":
                shape = tuple(alloc.tensor_shape)
                dtype = _mybir.dt.np(alloc.dtype)
                out_names.append(name)
                out_avals.append(jax.core.ShapedArray(shape, dtype))
                zero_outs.append((shape, dtype))
        part_name = nc.partition_id_tensor.name if nc.partition_id_tensor else None
        if part_name is not None:
            in_names = [n for n in in_names if n != part_name]
        self.in_names, self.out_names = in_names, out_names
        self.out_shapes = zero_outs
        n_params = len(in_names)
        all_names = in_names + out_names

        def _body(*args):
            operands = list(args)
            if part_name is not None:
                operands.append(bass2jax.partition_id_tensor())
            outs = bass2jax._bass_exec_p.bind(
                *operands,
                out_avals=tuple(out_avals),
                in_names=tuple(all_names + ([part_name] if part_name else [])),
                out_names=tuple(out_names),
                lowering_input_output_aliases=(),
                sim_require_finite=True,
                sim_require_nnan=True,
                nc=nc,
            )
            return tuple(outs)

        devices = jax.devices()[:NCORES]
        self.mesh = Mesh(np.asarray(devices), ("core",))
        n_out = len(out_names)
        in_specs = (PartitionSpec("core"),) * (n_params + n_out)
        out_specs = (PartitionSpec("core"),) * n_out
        donate = tuple(range(n_params, n_params + n_out))
        self.fn = jax.jit(
            shard_map(
                _body,
                mesh=self.mesh,
                in_specs=in_specs,
                out_specs=out_specs,
                check_rep=False,
            ),
            donate_argnums=donate,
            keep_unused=True,
        )

    def device_inputs(self, in_maps):
        """Concat per-core inputs on axis 0 and push to devices once."""
        import jax
        from jax.sharding import NamedSharding, PartitionSpec

        sh = NamedSharding(self.mesh, PartitionSpec("core"))
        arrs = []
        for name in self.in_names:
            cat = np.concatenate([np.asarray(m[name]) for m in in_maps], axis=0)
            arrs.append(jax.device_put(cat, sh))
        return arrs

    def device_zeros(self):
        import jax.numpy as jnp
        from jax.sharding import NamedSharding, PartitionSpec

        sh = NamedSharding(self.mesh, PartitionSpec("core"))
        zs = []
        for shape, dtype in self.out_shapes:
            gshape = (shape[0] * NCORES,) + tuple(shape[1:])
            zs.append(
                self.jax.jit(
                    lambda s=gshape, d=dtype: jnp.zeros(s, d), out_shardings=sh
                )()
            )
        return zs

    def __call__(self, dev_in, dev_zeros):
        outs = self.fn(*dev_in, *dev_zeros)
        return [np.asarray(o) for o in outs]

    def split_outputs(self, np_outs):
        results = [dict() for _ in range(NCORES)]
        for name, arr in zip(self.out_names, np_outs):
            per = arr.shape[0] // NCORES
            for c in range(NCORES):
                results[c][name] = arr[c * per : (c + 1) * per]
        return results


def get_runner():
    if "runner" not in _NC_CACHE:
        _NC_CACHE["runner"] = PjrtRunner(get_program())
    return _NC_CACHE["runner"]


def run(inputs):
    r = get_runner()
    in_maps = prep_inputs(**inputs)
    dev_in = r.device_inputs(in_maps)
    np_outs = r(dev_in, r.device_zeros())
    return assemble_output(r.split_outputs(np_outs)), r


def kernel(**inputs) -> np.ndarray:
    out, _ = run(inputs)
    return out


# revision 9
# speedup vs baseline: 296.3455x; 1.1792x over previous
"""DSAttention (de-stationary attention) Trainium2 Bass kernel.

Reference semantics (per batch b, head h):
    scores[l, s] = (q[l] . k[s]) * tau[b] + delta[b, s]
    attn = softmax(scale * scores, axis=s),  scale = 1/sqrt(D)
    out[l] = sum_s attn[l, s] * v[s]

Sharding: B*H = 32 (b,h) pairs split 4-per-core across 8 NeuronCores.
Host folds tau[b]*scale into Q and scale into delta, and pre-transposes
Q/K to [D, L] so the device only does contiguous DMA loads.

Device algorithm per (b,h)  (flash-style, no max subtraction needed:
|logits| <= ~10 so exp() is safe in fp32):
    scoresT[s_blk, l] = K^T_blk.T @ Q^T           (PE, f32r)
    P = exp(scoresT + delta[s])                   (ACT, fused bias)
    outT[65, l]   += V'_blk.T @ P_blk             (PE, f32r accumulate)
      where V' = [V | ones]  ->  row 64 of outT = softmax denominator
    out[l, d] = transpose(outT)[:, 0:64] * (1/transpose(outT)[:, 64])
"""

import sys
from contextlib import ExitStack

import numpy as np

sys.path.insert(0, "/opt/trn_rl_repo")

import concourse.bacc as bacc  # noqa: E402
import concourse.mybir as mybir  # noqa: E402
import concourse.tile as tile  # noqa: E402
from concourse import bass_utils  # noqa: E402

B, L, S, H, D = 4, 2048, 2048, 8, 64
NCORES = 8
BH = B * H
PER = BH // NCORES  # (b,h) pairs per core
SCALE = 1.0 / np.sqrt(D)
PB = 128  # partition block (s-chunk size)
F32 = mybir.dt.float32
F32R = mybir.dt.float32r
BF16 = mybir.dt.bfloat16

ActFn = mybir.ActivationFunctionType


def build_program(per=PER, l_len=L, s_len=S, d=D, w=1024, reps=1, mmdt="f32r"):
    """Build the per-core Bass program. w = l-pass width (free-dim chunk)."""
    assert l_len % w == 0 and w % 512 == 0 and s_len % PB == 0
    MDT = {"f32r": F32R, "bf16": BF16}[mmdt]
    ns = s_len // PB  # number of s-chunks
    npass = l_len // w  # l passes
    nj = w // 512  # 512-wide matmul slices per pass

    nc = bacc.Bacc("TRN2", target_bir_lowering=False, debug=False)

    qT_d = nc.dram_tensor("qT", [per, d, l_len], MDT, kind="ExternalInput").ap()
    kT_d = nc.dram_tensor("kT", [per, d, s_len], MDT, kind="ExternalInput").ap()
    v_d = nc.dram_tensor("v", [per, s_len, d + 1], MDT, kind="ExternalInput").ap()
    dl_d = nc.dram_tensor("dl", [per, s_len], F32, kind="ExternalInput").ap()
    id_d = nc.dram_tensor("ident", [d + 1, d + 1], F32, kind="ExternalInput").ap()
    o_d = nc.dram_tensor("out", [per, l_len, d], F32, kind="ExternalOutput").ap()

    with tile.TileContext(nc) as tc, ExitStack() as ctx:
        const_pool = ctx.enter_context(tc.tile_pool(name="const", bufs=1))
        in_pool = ctx.enter_context(tc.tile_pool(name="inp", bufs=2))
        p_pool = ctx.enter_context(tc.tile_pool(name="pp", bufs=3))
        ot_pool = ctx.enter_context(tc.tile_pool(name="otp", bufs=2))
        os_pool = ctx.enter_context(tc.tile_pool(name="osp", bufs=3))
        rc_pool = ctx.enter_context(tc.tile_pool(name="rcp", bufs=3))
        qk_psum = ctx.enter_context(tc.tile_pool(name="qkp", bufs=2, space="PSUM"))
        pv_psum = ctx.enter_context(tc.tile_pool(name="pvp", bufs=nj, space="PSUM"))
        tr_psum = ctx.enter_context(tc.tile_pool(name="trp", bufs=2, space="PSUM"))

        ident = const_pool.tile([d + 1, d + 1], F32)
        nc.sync.dma_start(ident[:], id_d[:])

        rep_ctx = (
            tc.For_i(0, reps, 1, hint_engines=(mybir.EngineType.PE,))
            if reps > 1
            else None
        )
        if rep_ctx is not None:
            ctx.enter_context(rep_ctx)
        for i in range(per):
            qT = in_pool.tile([d, l_len], MDT, tag="qT")
            nc.sync.dma_start(qT[:], qT_d[i])
            kT = in_pool.tile([d, s_len], MDT, tag="kT")
            nc.sync.dma_start(kT[:], kT_d[i])

            # V' = [V | ones] (ones appended host-side) as [128, ns*(d+1)]
            vv = in_pool.tile([PB, ns * (d + 1)], MDT, tag="vv")
            vv3 = vv[:].rearrange("p (c e) -> p c e", e=d + 1)
            nc.sync.dma_start(
                vv3[:], v_d[i].rearrange("(c p) e -> p c e", p=PB)
            )

            # delta (pre-scaled) as per-partition bias columns [128, ns]
            dl = in_pool.tile([PB, ns], F32, tag="dl")
            nc.sync.dma_start(dl[:], dl_d[i].rearrange("(c p) -> p c", p=PB))

            for lp in range(npass):
                l0 = lp * w
                pvs = [
                    pv_psum.tile([d + 1, 512], F32, tag="pv", name=f"pv_{i}_{lp}_{j}")
                    for j in range(nj)
                ]
                for s in range(ns):
                    qk = qk_psum.tile([PB, w], F32, tag="qk")
                    for j in range(nj):
                        nc.tensor.matmul(
                            qk[:, j * 512 : (j + 1) * 512],
                            kT[:, s * PB : (s + 1) * PB],
                            qT[:, l0 + j * 512 : l0 + (j + 1) * 512],
                            start=True,
                            stop=True,
                        )
                    pt = p_pool.tile([PB, w], MDT, tag="p")
                    nc.scalar.activation(
                        pt[:], qk[:], ActFn.Exp, bias=dl[:, s : s + 1], scale=1.0
                    )
                    for j in range(nj):
                        nc.tensor.matmul(
                            pvs[j][:],
                            vv[:, s * (d + 1) : (s + 1) * (d + 1)],
                            pt[:, j * 512 : (j + 1) * 512],
                            start=(s == 0),
                            stop=(s == ns - 1),
                        )
                # epilogue: transpose [65, 128] blocks -> [128, 65], normalize
                ot = ot_pool.tile([d + 1, w], F32, tag="ot")
                for j in range(nj):
                    nc.vector.tensor_copy(ot[:, j * 512 : (j + 1) * 512], pvs[j][:])
                for j in range(w // PB):
                    tr = tr_psum.tile([PB, d + 1], F32, tag="tr")
                    nc.tensor.transpose(
                        tr[:], ot[:, j * PB : (j + 1) * PB], ident[:]
                    )
                    rc = rc_pool.tile([PB, 1], F32, tag="rc")
                    nc.vector.reciprocal(rc[:], tr[:, d : d + 1])
                    ob = os_pool.tile([PB, d], F32, tag="ob")
                    nc.vector.tensor_scalar_mul(ob[:], tr[:, 0:d], rc[:])
                    nc.sync.dma_start(
                        o_d[i, l0 + j * PB : l0 + (j + 1) * PB, :], ob[:]
                    )

    nc.compile()
    return nc


def prep_inputs(queries, keys, values, tau, delta, mmdt="f32r"):
    """Host-side shard prep: fold tau*scale into Q, scale delta, transpose
    Q/K to [D, L] layout, split (b,h) pairs across cores."""
    q = np.asarray(queries, dtype=np.float32)
    k = np.asarray(keys, dtype=np.float32)
    v = np.asarray(values, dtype=np.float32)
    tau = np.asarray(tau, dtype=np.float32)
    delta = np.asarray(delta, dtype=np.float32)

    qs = q * (SCALE * tau)[:, None, None, None]  # [B, L, H, D]
    qT = np.ascontiguousarray(qs.transpose(0, 2, 3, 1)).reshape(BH, D, L)
    kT = np.ascontiguousarray(k.transpose(0, 2, 3, 1)).reshape(BH, D, S)
    vb = np.ascontiguousarray(v.transpose(0, 2, 1, 3)).reshape(BH, S, D)
    vb = np.concatenate([vb, np.ones((BH, S, 1), dtype=np.float32)], axis=2)
    dls = (SCALE * delta).astype(np.float32)  # [B, S]
    ident = np.eye(D + 1, dtype=np.float32)
    if mmdt == "bf16":
        import ml_dtypes

        qT = qT.astype(ml_dtypes.bfloat16)
        kT = kT.astype(ml_dtypes.bfloat16)
        vb = vb.astype(ml_dtypes.bfloat16)

    in_maps = []
    for c in range(NCORES):
        sl = slice(c * PER, (c + 1) * PER)
        bh_idx = np.arange(c * PER, (c + 1) * PER)
        in_maps.append(
            {
                "qT": np.ascontiguousarray(qT[sl]),
                "kT": np.ascontiguousarray(kT[sl]),
                "v": np.ascontiguousarray(vb[sl]),
                "dl": np.ascontiguousarray(dls[bh_idx // H]),
                "ident": ident,
            }
        )
    return in_maps


def assemble_output(results):
    out_bh = np.concatenate([r["out"] for r in results], axis=0)  # [32, L, D]
    out = out_bh.reshape(B, H, L, D).transpose(0, 2, 1, 3)  # [B, L, H, D]
    return np.ascontiguousarray(out).astype(np.float32)


_NC_CACHE = {}


def get_program():
    if "nc" not in _NC_CACHE:
        _NC_CACHE["nc"] = build_program()
    return _NC_CACHE["nc"]


class PjrtRunner:
    """Cached shard_map runner mirroring bass2jax.run_bass_via_pjrt, but
    reusable across calls (single jit closure) so repeat executions skip
    retracing and input re-transfer (for benchmarking)."""

    def __init__(self, nc):
        import jax
        from jax.sharding import Mesh, PartitionSpec
        from jax.experimental.shard_map import shard_map
        from concourse import bass2jax, mybir as _mybir

        bass2jax.install_neuronx_cc_hook()
        self.nc = nc
        self.jax = jax
        in_names, out_names, out_avals, zero_outs = [], [], [], []
        for alloc in nc.m.functions[0].allocations:
            if not isinstance(alloc, _mybir.MemoryLocationSet):
                continue
            name = alloc.memorylocations[0].name
            if alloc.kind == "ExternalInput":
                in_names.append(name)
            elif alloc.kind == "ExternalOutput":
                shape = tuple(alloc.tensor_shape)
                dtype = _mybir.dt.np(alloc.dtype)
                out_names.append(name)
                out_avals.append(jax.core.ShapedArray(shape, dtype))
                zero_outs.append((shape, dtype))
        part_name = nc.partition_id_tensor.name if nc.partition_id_tensor else None
        if part_name is not None:
            in_names = [n for n in in_names if n != part_name]
        self.in_names, self.out_names = in_names, out_names
        self.out_shapes = zero_outs
        n_params = len(in_names)
        all_names = in_names + out_names

        def _body(*args):
            operands = list(args)
            if part_name is not None:
                operands.append(bass2jax.partition_id_tensor())
            outs = bass2jax._bass_exec_p.bind(
                *operands,
                out_avals=tuple(out_avals),
                in_names=tuple(all_names + ([part_name] if part_name else [])),
                out_names=tuple(out_names),
                lowering_input_output_aliases=(),
                sim_require_finite=True,
                sim_require_nnan=True,
                nc=nc,
            )
            return tuple(outs)

        devices = jax.devices()[:NCORES]
        self.mesh = Mesh(np.asarray(devices), ("core",))
        n_out = len(out_names)
        in_specs = (PartitionSpec("core"),) * (n_params + n_out)
        out_specs = (PartitionSpec("core"),) * n_out
        donate = tuple(range(n_params, n_params + n_out))
        self.fn = jax.jit(
            shard_map(
                _body,
                mesh=self.mesh,
                in_specs=in_specs,
                out_specs=out_specs,
                check_rep=False,
            ),
            donate_argnums=donate,
            keep_unused=True,
        )

    def device_inputs(self, in_maps):
        """Concat per-core inputs on axis 0 and push to devices once."""
        import jax
        from jax.sharding import NamedSharding, PartitionSpec

        sh = NamedSharding(self.mesh, PartitionSpec("core"))
        arrs = []
        for name in self.in_names:
            cat = np.concatenate([np.asarray(m[name]) for m in in_maps], axis=0)
            arrs.append(jax.device_put(cat, sh))
        return arrs

    def device_zeros(self):
        import jax.numpy as jnp
        from jax.sharding import NamedSharding, PartitionSpec

        sh = NamedSharding(self.mesh, PartitionSpec("core"))
        zs = []
        for shape, dtype in self.out_shapes:
            gshape = (shape[0] * NCORES,) + tuple(shape[1:])
            zs.append(
                self.jax.jit(
                    lambda s=gshape, d=dtype: jnp.zeros(s, d), out_shardings=sh
                )()
            )
        return zs

    def __call__(self, dev_in, dev_zeros):
        outs = self.fn(*dev_in, *dev_zeros)
        return [np.asarray(o) for o in outs]

    def split_outputs(self, np_outs):
        results = [dict() for _ in range(NCORES)]
        for name, arr in zip(self.out_names, np_outs):
            per = arr.shape[0] // NCORES
            for c in range(NCORES):
                results[c][name] = arr[c * per : (c + 1) * per]
        return results


def get_runner():
    if "runner" not in _NC_CACHE:
        _NC_CACHE["runner"] = PjrtRunner(get_program())
    return _NC_CACHE["runner"]


def run(inputs):
    r = get_runner()
    in_maps = prep_inputs(**inputs)
    dev_in = r.device_inputs(in_maps)
    np_outs = r(dev_in, r.device_zeros())
    return assemble_output(r.split_outputs(np_outs)), r


def kernel(**inputs) -> np.ndarray:
    out, _ = run(inputs)
    return out


# revision 10
# speedup vs baseline: 301.2716x; 1.0166x over previous
"""DSAttention (de-stationary attention) Trainium2 Bass kernel.

Reference semantics (per batch b, head h):
    scores[l, s] = (q[l] . k[s]) * tau[b] + delta[b, s]
    attn = softmax(scale * scores, axis=s),  scale = 1/sqrt(D)
    out[l] = sum_s attn[l, s] * v[s]

Sharding: B*H = 32 (b,h) pairs split 4-per-core across 8 NeuronCores.
Host folds tau[b]*scale into Q and scale into delta, and pre-transposes
Q/K to [D, L] so the device only does contiguous DMA loads.

Device algorithm per (b,h)  (flash-style, no max subtraction needed:
|logits| <= ~10 so exp() is safe in fp32):
    scoresT[s_blk, l] = K^T_blk.T @ Q^T           (PE, f32r)
    P = exp(scoresT + delta[s])                   (ACT, fused bias)
    outT[65, l]   += V'_blk.T @ P_blk             (PE, f32r accumulate)
      where V' = [V | ones]  ->  row 64 of outT = softmax denominator
    out[l, d] = transpose(outT)[:, 0:64] * (1/transpose(outT)[:, 64])
"""

import sys
from contextlib import ExitStack

import numpy as np

sys.path.insert(0, "/opt/trn_rl_repo")

import concourse.bacc as bacc  # noqa: E402
import concourse.mybir as mybir  # noqa: E402
import concourse.tile as tile  # noqa: E402
from concourse import bass_utils  # noqa: E402

B, L, S, H, D = 4, 2048, 2048, 8, 64
NCORES = 8
BH = B * H
PER = BH // NCORES  # (b,h) pairs per core
SCALE = 1.0 / np.sqrt(D)
PB = 128  # partition block (s-chunk size)
F32 = mybir.dt.float32
F32R = mybir.dt.float32r
BF16 = mybir.dt.bfloat16

ActFn = mybir.ActivationFunctionType


def build_program(per=PER, l_len=L, s_len=S, d=D, w=1024, reps=1, mmdt="f32r"):
    """Build the per-core Bass program. w = l-pass width (free-dim chunk)."""
    assert l_len % w == 0 and w % 512 == 0 and s_len % PB == 0
    MDT = {"f32r": F32R, "bf16": BF16}[mmdt]
    ns = s_len // PB  # number of s-chunks
    npass = l_len // w  # l passes
    nj = w // 512  # 512-wide matmul slices per pass

    nc = bacc.Bacc("TRN2", target_bir_lowering=False, debug=False)

    qT_d = nc.dram_tensor("qT", [per, d, l_len], MDT, kind="ExternalInput").ap()
    kT_d = nc.dram_tensor("kT", [per, d, s_len], MDT, kind="ExternalInput").ap()
    v_d = nc.dram_tensor("v", [per, s_len, d + 1], MDT, kind="ExternalInput").ap()
    dl_d = nc.dram_tensor("dl", [per, s_len], F32, kind="ExternalInput").ap()
    id_d = nc.dram_tensor("ident", [d + 1, d + 1], F32, kind="ExternalInput").ap()
    o_d = nc.dram_tensor("out", [per, l_len, d], F32, kind="ExternalOutput").ap()

    with tile.TileContext(nc) as tc, ExitStack() as ctx:
        const_pool = ctx.enter_context(tc.tile_pool(name="const", bufs=1))
        in_pool = ctx.enter_context(tc.tile_pool(name="inp", bufs=2))
        p_pool = ctx.enter_context(tc.tile_pool(name="pp", bufs=3))
        ot_pool = ctx.enter_context(tc.tile_pool(name="otp", bufs=2))
        os_pool = ctx.enter_context(tc.tile_pool(name="osp", bufs=3))
        rc_pool = ctx.enter_context(tc.tile_pool(name="rcp", bufs=3))
        qk_psum = ctx.enter_context(tc.tile_pool(name="qkp", bufs=2, space="PSUM"))
        pv_psum = ctx.enter_context(tc.tile_pool(name="pvp", bufs=nj, space="PSUM"))
        tr_psum = ctx.enter_context(tc.tile_pool(name="trp", bufs=2, space="PSUM"))

        ident = const_pool.tile([d + 1, d + 1], F32)
        nc.sync.dma_start(ident[:], id_d[:])

        rep_ctx = (
            tc.For_i(0, reps, 1, hint_engines=(mybir.EngineType.PE,))
            if reps > 1
            else None
        )
        if rep_ctx is not None:
            ctx.enter_context(rep_ctx)
        for i in range(per):
            qT = in_pool.tile([d, l_len], MDT, tag="qT")
            nc.sync.dma_start(qT[:], qT_d[i])
            kT = in_pool.tile([d, s_len], MDT, tag="kT")
            nc.sync.dma_start(kT[:], kT_d[i])

            # V' = [V | ones] (ones appended host-side) as [128, ns*(d+1)]
            vv = in_pool.tile([PB, ns * (d + 1)], MDT, tag="vv")
            vv3 = vv[:].rearrange("p (c e) -> p c e", e=d + 1)
            nc.sync.dma_start(
                vv3[:], v_d[i].rearrange("(c p) e -> p c e", p=PB)
            )

            # delta (pre-scaled) as per-partition bias columns [128, ns]
            dl = in_pool.tile([PB, ns], F32, tag="dl")
            nc.sync.dma_start(dl[:], dl_d[i].rearrange("(c p) -> p c", p=PB))

            for lp in range(npass):
                l0 = lp * w
                pvs = [
                    pv_psum.tile([d + 1, 512], F32, tag="pv", name=f"pv_{i}_{lp}_{j}")
                    for j in range(nj)
                ]
                def emit_qk(s):
                    qk = qk_psum.tile(
                        [PB, w], F32, tag="qk", name=f"qk_{i}_{lp}_{s}"
                    )
                    for j in range(nj):
                        nc.tensor.matmul(
                            qk[:, j * 512 : (j + 1) * 512],
                            kT[:, s * PB : (s + 1) * PB],
                            qT[:, l0 + j * 512 : l0 + (j + 1) * 512],
                            start=True,
                            stop=True,
                        )
                    return qk

                # software pipeline: emit QK[s+1] before PV[s] so the PE's
                # in-order queue never blocks ACT[s+1] behind PV[s]'s wait
                # on ACT[s].
                qk_cur = emit_qk(0)
                for s in range(ns):
                    pt = p_pool.tile([PB, w], MDT, tag="p")
                    nc.scalar.activation(
                        pt[:], qk_cur[:], ActFn.Exp, bias=dl[:, s : s + 1], scale=1.0
                    )
                    if s + 1 < ns:
                        qk_cur = emit_qk(s + 1)
                    for j in range(nj):
                        nc.tensor.matmul(
                            pvs[j][:],
                            vv[:, s * (d + 1) : (s + 1) * (d + 1)],
                            pt[:, j * 512 : (j + 1) * 512],
                            start=(s == 0),
                            stop=(s == ns - 1),
                        )
                # epilogue: transpose [65, 128] blocks -> [128, 65], normalize
                ot = ot_pool.tile([d + 1, w], F32, tag="ot")
                for j in range(nj):
                    nc.vector.tensor_copy(ot[:, j * 512 : (j + 1) * 512], pvs[j][:])
                for j in range(w // PB):
                    tr = tr_psum.tile([PB, d + 1], F32, tag="tr")
                    nc.tensor.transpose(
                        tr[:], ot[:, j * PB : (j + 1) * PB], ident[:]
                    )
                    rc = rc_pool.tile([PB, 1], F32, tag="rc")
                    nc.vector.reciprocal(rc[:], tr[:, d : d + 1])
                    ob = os_pool.tile([PB, d], F32, tag="ob")
                    nc.vector.tensor_scalar_mul(ob[:], tr[:, 0:d], rc[:])
                    nc.sync.dma_start(
                        o_d[i, l0 + j * PB : l0 + (j + 1) * PB, :], ob[:]
                    )

    nc.compile()
    return nc


def prep_inputs(queries, keys, values, tau, delta, mmdt="f32r"):
    """Host-side shard prep: fold tau*scale into Q, scale delta, transpose
    Q/K to [D, L] layout, split (b,h) pairs across cores."""
    q = np.asarray(queries, dtype=np.float32)
    k = np.asarray(keys, dtype=np.float32)
    v = np.asarray(values, dtype=np.float32)
    tau = np.asarray(tau, dtype=np.float32)
    delta = np.asarray(delta, dtype=np.float32)

    qs = q * (SCALE * tau)[:, None, None, None]  # [B, L, H, D]
    qT = np.ascontiguousarray(qs.transpose(0, 2, 3, 1)).reshape(BH, D, L)
    kT = np.ascontiguousarray(k.transpose(0, 2, 3, 1)).reshape(BH, D, S)
    vb = np.ascontiguousarray(v.transpose(0, 2, 1, 3)).reshape(BH, S, D)
    vb = np.concatenate([vb, np.ones((BH, S, 1), dtype=np.float32)], axis=2)
    dls = (SCALE * delta).astype(np.float32)  # [B, S]
    ident = np.eye(D + 1, dtype=np.float32)
    if mmdt == "bf16":
        import ml_dtypes

        qT = qT.astype(ml_dtypes.bfloat16)
        kT = kT.astype(ml_dtypes.bfloat16)
        vb = vb.astype(ml_dtypes.bfloat16)

    in_maps = []
    for c in range(NCORES):
        sl = slice(c * PER, (c + 1) * PER)
        bh_idx = np.arange(c * PER, (c + 1) * PER)
        in_maps.append(
            {
                "qT": np.ascontiguousarray(qT[sl]),
                "kT": np.ascontiguousarray(kT[sl]),
                "v": np.ascontiguousarray(vb[sl]),
                "dl": np.ascontiguousarray(dls[bh_idx // H]),
                "ident": ident,
            }
        )
    return in_maps


def assemble_output(results):
    out_bh = np.concatenate([r["out"] for r in results], axis=0)  # [32, L, D]
    out = out_bh.reshape(B, H, L, D).transpose(0, 2, 1, 3)  # [B, L, H, D]
    return np.ascontiguousarray(out).astype(np.float32)


_NC_CACHE = {}


def get_program():
    if "nc" not in _NC_CACHE:
        _NC_CACHE["nc"] = build_program()
    return _NC_CACHE["nc"]


class PjrtRunner:
    """Cached shard_map runner mirroring bass2jax.run_bass_via_pjrt, but
    reusable across calls (single jit closure) so repeat executions skip
    retracing and input re-transfer (for benchmarking)."""

    def __init__(self, nc):
        import jax
        from jax.sharding import Mesh, PartitionSpec
        from jax.experimental.shard_map import shard_map
        from concourse import bass2jax, mybir as _mybir

        bass2jax.install_neuronx_cc_hook()
        self.nc = nc
        self.jax = jax
        in_names, out_names, out_avals, zero_outs = [], [], [], []
        for alloc in nc.m.functions[0].allocations:
            if not isinstance(alloc, _mybir.MemoryLocationSet):
                continue
            name = alloc.memorylocations[0].name
            if alloc.kind == "ExternalInput":
                in_names.append(name)
            elif alloc.kind == "ExternalOutput":
                shape = tuple(alloc.tensor_shape)
                dtype = _mybir.dt.np(alloc.dtype)
                out_names.append(name)
                out_avals.append(jax.core.ShapedArray(shape, dtype))
                zero_outs.append((shape, dtype))
        part_name = nc.partition_id_tensor.name if nc.partition_id_tensor else None
        if part_name is not None:
            in_names = [n for n in in_names if n != part_name]
        self.in_names, self.out_names = in_names, out_names
        self.out_shapes = zero_outs
        n_params = len(in_names)
        all_names = in_names + out_names

        def _body(*args):
            operands = list(args)
            if part_name is not None:
                operands.append(bass2jax.partition_id_tensor())
            outs = bass2jax._bass_exec_p.bind(
                *operands,
                out_avals=tuple(out_avals),
                in_names=tuple(all_names + ([part_name] if part_name else [])),
                out_names=tuple(out_names),
                lowering_input_output_aliases=(),
                sim_require_finite=True,
                sim_require_nnan=True,
                nc=nc,
            )
            return tuple(outs)

        devices = jax.devices()[:NCORES]
        self.mesh = Mesh(np.asarray(devices), ("core",))
        n_out = len(out_names)
        in_specs = (PartitionSpec("core"),) * (n_params + n_out)
        out_specs = (PartitionSpec("core"),) * n_out
        donate = tuple(range(n_params, n_params + n_out))
        self.fn = jax.jit(
            shard_map(
                _body,
                mesh=self.mesh,
                in_specs=in_specs,
                out_specs=out_specs,
                check_rep=False,
            ),
            donate_argnums=donate,
            keep_unused=True,
        )

    def device_inputs(self, in_maps):
        """Concat per-core inputs on axis 0 and push to devices once."""
        import jax
        from jax.sharding import NamedSharding, PartitionSpec

        sh = NamedSharding(self.mesh, PartitionSpec("core"))
        arrs = []
        for name in self.in_names:
            cat = np.concatenate([np.asarray(m[name]) for m in in_maps], axis=0)
            arrs.append(jax.device_put(cat, sh))
        return arrs

    def device_zeros(self):
        import jax.numpy as jnp
        from jax.sharding import NamedSharding, PartitionSpec

        sh = NamedSharding(self.mesh, PartitionSpec("core"))
        zs = []
        for shape, dtype in self.out_shapes:
            gshape = (shape[0] * NCORES,) + tuple(shape[1:])
            zs.append(
                self.jax.jit(
                    lambda s=gshape, d=dtype: jnp.zeros(s, d), out_shardings=sh
                )()
            )
        return zs

    def __call__(self, dev_in, dev_zeros):
        outs = self.fn(*dev_in, *dev_zeros)
        return [np.asarray(o) for o in outs]

    def split_outputs(self, np_outs):
        results = [dict() for _ in range(NCORES)]
        for name, arr in zip(self.out_names, np_outs):
            per = arr.shape[0] // NCORES
            for c in range(NCORES):
                results[c][name] = arr[c * per : (c + 1) * per]
        return results


def get_runner():
    if "runner" not in _NC_CACHE:
        _NC_CACHE["runner"] = PjrtRunner(get_program())
    return _NC_CACHE["runner"]


def run(inputs):
    r = get_runner()
    in_maps = prep_inputs(**inputs)
    dev_in = r.device_inputs(in_maps)
    np_outs = r(dev_in, r.device_zeros())
    return assemble_output(r.split_outputs(np_outs)), r


def kernel(**inputs) -> np.ndarray:
    out, _ = run(inputs)
    return out
